# revision 38
# baseline (speedup 1.0000x reference)
"""MoE layer (top-2 routing, 8 experts) on 8 Trainium2 NeuronCores.

Strategy — expert-parallel with hidden-dim (H) slicing for perfect balance:
  - Host computes the gate (router math in fp64 numpy): logits, top-2 experts
    per token, softmax gates; tokens are sorted into per-expert segments.
  - ReLU is elementwise in H, so each expert MLP decomposes exactly into 8
    independent H-slice MLPs (D x 512 x D). Core c holds slice c of EVERY
    expert (same 16.8MB fp16 weight footprint as one whole expert).
  - The kernel runs 8 passes; pass e = all 8 cores compute expert e's slice
    over exactly n_e tokens (identical shapes on every core -> SPMD, zero
    padding, perfect load balance).
  - Each core emits gate-weighted partial outputs; host sums the 8 cores'
    partials and scatter-adds each token's two expert contributions.

Tail scheduling (from profile analysis): the final tile is 128 tokens and
the last two tiles' y DMAs ride scalar then sync instead of gpsimd, so
gpsimd's slow queue-drain (several us after its last transfer) completes
while the PE is still computing, and only ~0.26MB trails the last matmul.
Gates are fp16 (halves that stream; error budget is 2e-2, measured ~5e-4).

Hardcoded problem shape: x(8192,1024) w1(8,1024,4096) w2(8,4096,1024).
"""

import numpy as np

import concourse.tile as tile
import concourse.mybir as mybir
from concourse import bacc
from concourse.bass_utils import run_bass_kernel_spmd

E = 8          # experts
D = 1024       # model dim
H = 4096       # hidden dim
HS = H // 8    # per-core hidden slice (512)
NHS = HS // 128  # h-tiles per slice (4)
TOP_K = 2
N_CORES = 8
ND = D // 128   # 8 d-tiles

F32 = mybir.dt.float32
F16 = mybir.dt.float16


def _balanced_tiles(start, n, max_tile=512):
    """Split [start, start+n) into ceil(n/max_tile) near-equal tiles."""
    nt = max(1, -(-n // max_tile))
    base, rem = divmod(n, nt)
    tiles = []
    t = start
    for i in range(nt):
        sz = base + (1 if i < rem else 0)
        tiles.append((t, sz))
        t += sz
    return tiles


def build_moe(counts):
    """Build + compile the 8-pass H-sliced expert MLP program.

    counts: per-expert token counts (same on every core; pass e covers
    exactly counts[e] tokens). Weight/x/g/y DRAM tensors hold the per-core
    slice data laid out expert-major (see moe_run for host layouts).
    """
    total = int(sum(counts))
    starts = np.concatenate([[0], np.cumsum(counts)]).astype(int)

    nc = bacc.Bacc("TRN2", target_bir_lowering=False, debug=False, num_devices=N_CORES)

    xt = nc.dram_tensor("xt", [D, total], F16, kind="ExternalInput")   # sorted x^T
    w1 = nc.dram_tensor("w1", [D, E * HS], F16, kind="ExternalInput")  # cols e*512..: this core's slice of expert e
    w2 = nc.dram_tensor("w2", [E * HS, D], F16, kind="ExternalInput")  # rows e*512..: this core's slice of expert e
    b1 = nc.dram_tensor("b1", [128, E * NHS], F32, kind="ExternalInput")
    g = nc.dram_tensor("g", [128, total], F16, kind="ExternalInput")   # gates, replicated rows
    yt = nc.dram_tensor("yt", [D, total], F16, kind="ExternalOutput")

    xt_ap, w1_ap, w2_ap, b1_ap, g_ap, yt_ap = (
        t.ap() for t in (xt, w1, w2, b1, g, yt)
    )

    with tile.TileContext(nc) as tc:
        with (
            tc.tile_pool(name="wpool", bufs=1) as wpool,
            tc.tile_pool(name="xpool", bufs=3) as xpool,
            tc.tile_pool(name="hpool", bufs=10) as hpool,
            tc.tile_pool(name="ypool", bufs=8) as ypool,
            tc.tile_pool(name="gpool", bufs=4) as gpool,
            tc.tile_pool(name="ph", bufs=4, space="PSUM") as ph_pool,
            tc.tile_pool(name="py", bufs=4, space="PSUM") as py_pool,
        ):
            def load_gate(t0, tn):
                g_sb = gpool.tile([128, 512], F16, name=f"gsb{t0}", tag="gsb")
                nc.sync.dma_start(g_sb[:, :tn], g_ap[:, t0:t0 + tn])
                return g_sb

            def load_tok_tile(t0, tn, split_first=False):
                # One DMA moves all 8 d-slices of this token tile into a wide
                # tile (d-slice j at columns [j*tn, (j+1)*tn)).
                xtile = xpool.tile([128, ND * 512], F16, name=f"xsb{t0}", tag="xsb")
                if split_first:
                    nc.sync.dma_start(xtile[:, :tn], xt_ap[0:128, t0:t0 + tn])
                    src = xt_ap[128:, t0:t0 + tn].rearrange("(dd p) t -> p dd t", p=128)
                    dst = xtile[:, tn:ND * tn].rearrange("p (dd t) -> p dd t", t=tn)
                    nc.sync.dma_start(dst, src)
                else:
                    # Two half-tile DMAs: the tile's first matmul then waits
                    # only on d-chunks 0-3 (region-granular deps), not the
                    # whole 1MB transfer.
                    h8 = ND // 2
                    src = xt_ap[:h8 * 128, t0:t0 + tn].rearrange("(dd p) t -> p dd t", p=128)
                    dst = xtile[:, :h8 * tn].rearrange("p (dd t) -> p dd t", t=tn)
                    nc.sync.dma_start(dst, src)
                    src2 = xt_ap[h8 * 128:, t0:t0 + tn].rearrange("(dd p) t -> p dd t", p=128)
                    dst2 = xtile[:, h8 * tn:ND * tn].rearrange("p (dd t) -> p dd t", t=tn)
                    nc.sync.dma_start(dst2, src2)
                return [xtile[:, d * tn:(d + 1) * tn] for d in range(ND)]

            # PE warm-up: dummy matmuls on a zeroed tile cover the initial DMA
            # wait and ramp the clock to full pstate before the real stream.
            warm = wpool.tile([128, 512], F16, name="warm", tag="warm")
            nc.vector.memset(warm[:], 0.0)
            warm_ps = ph_pool.tile([128, 512], F32, name="warmps", tag="ph")
            for _ in range(30):
                nc.tensor.matmul(warm_ps[:], warm[:, :128], warm[:], start=True, stop=True)

            pass_tiles = []
            for e in range(E):
                if e == E - 1 and counts[e] >= 768:
                    # Small final tile: shortens the post-last-matmul chain.
                    tl = _balanced_tiles(starts[e], counts[e] - 128)
                    tl.append((starts[e] + counts[e] - 128, 128))
                else:
                    tl = _balanced_tiles(starts[e], counts[e])
                pass_tiles.append(tl)
            n_tiles_total = sum(len(t) for t in pass_tiles)

            # Startup stream on sync, in strict consumption order. Pass 0
            # only needs expert 0's half of the q0 weight chunks, so the q0
            # DMAs are split per-expert: the critical prefix (x0 + w1-e0) is
            # 2.15MB instead of 4.25MB and real matmuls start ~5us earlier.
            prefetched = {pass_tiles[0][0][0]: load_tok_tile(*pass_tiles[0][0], split_first=True)}
            g_prefetched = {pass_tiles[0][0][0]: load_gate(*pass_tiles[0][0])}
            b1_sb = wpool.tile([128, E * NHS], F32, name="b1sb", tag="b1sb")
            nc.sync.dma_start(b1_sb[:], b1_ap[:, :])

            w1_sb = [[None] * E for _ in range(ND)]  # [d][e] -> [128, HS]
            w1_dmas = [[] for _ in range(E // 2)]
            w1q0_tiles = []
            for d in range(ND):
                t = wpool.tile([128, 2 * HS], F16, name=f"w1c{d}_0", tag=f"w1c{d}_0")
                w1q0_tiles.append(t)
                w1_sb[d][0] = t[:, :HS]
                w1_sb[d][1] = t[:, HS:]
            for d in range(ND):
                w1_dmas[0].append(nc.sync.dma_start(
                    w1q0_tiles[d][:, :HS], w1_ap[d * 128:(d + 1) * 128, 0:HS]))
            t1 = pass_tiles[0][1][0]
            prefetched[t1] = load_tok_tile(*pass_tiles[0][1])
            g_prefetched[t1] = load_gate(*pass_tiles[0][1])
            for d in range(ND):
                w1_dmas[0].append(nc.sync.dma_start(
                    w1q0_tiles[d][:, HS:], w1_ap[d * 128:(d + 1) * 128, HS:2 * HS]))

            # Later w1 groups as [128, 1024] pair chunks (2KB DMA lines),
            # dep-gated to stream during earlier passes.
            for q in range(1, E // 2):
                for d in range(ND):
                    t = wpool.tile([128, 2 * HS], F16, name=f"w1c{d}_{q}", tag=f"w1c{d}_{q}")
                    w1_dmas[q].append(nc.sync.dma_start(
                        t[:], w1_ap[d * 128:(d + 1) * 128, q * 2 * HS:(q + 1) * 2 * HS]
                    ))
                    w1_sb[d][2 * q] = t[:, :HS]
                    w1_sb[d][2 * q + 1] = t[:, HS:]

            # w2: one [128, 4*D] pack per expert on the idle Scalar queue,
            # dep-gated progressively (pack e released by an early pass-(e-1)
            # evac) so the 8.4MB stream doesn't contend with startup loads.
            w2_sb = []
            w2_dmas = []
            for e in range(E):
                t = wpool.tile([128, NHS * D], F16, name=f"w2p{e}", tag=f"w2p{e}")
                src = w2_ap[e * HS:(e + 1) * HS, :].rearrange("(ho p) d -> p ho d", p=128)
                dst = t.rearrange("p (ho d) -> p ho d", d=D)
                w2_dmas.append(nc.scalar.dma_start(dst, src))
                w2_sb.append(t)

            tile_idx = 0
            n_y = 0

            for e in range(E):
                for ti, (t0, tn) in enumerate(pass_tiles[e]):
                    x_sb = prefetched.pop(t0) if t0 in prefetched else load_tok_tile(t0, tn)
                    g_sb = g_prefetched.pop(t0) if t0 in g_prefetched else load_gate(t0, tn)

                    # Layer 1: H-slice^T[j] = relu(sum_d W1s[d, j]^T X^T[d] + b1s[j])
                    h_sb = []
                    for j in range(NHS):
                        ph = ph_pool.tile([128, 512], F32, name=f"ph{e}_{t0}_{j}", tag="ph")
                        for d in range(ND):
                            nc.tensor.matmul(
                                ph[:, :tn],
                                w1_sb[d][e][:, j * 128:(j + 1) * 128],
                                x_sb[d][:, :tn],
                                start=(d == 0),
                                stop=(d == ND - 1),
                            )
                        ht = hpool.tile([128, 512], F16, name=f"hsb{e}_{t0}_{j}", tag="hsb")
                        evac = nc.vector.tensor_scalar(
                            ht[:, :tn], ph[:, :tn],
                            b1_sb[:, e * NHS + j:e * NHS + j + 1], 0.0,
                            op0=mybir.AluOpType.add, op1=mybir.AluOpType.max,
                        )
                        if ti == 0 and j == 0:
                            if e + 1 < E:
                                tile.add_dep_helper(w2_dmas[e + 1].ins, evac.ins, sync=True,
                                                    reason="w2 prefetch spread across passes")
                            # w1 chunk group q feeds passes 2q/2q+1; release it
                            # one pass-pair early so weight DMA bandwidth is
                            # spread across the run instead of the startup.
                            if e % 2 == 0 and e // 2 + 1 < E // 2:
                                for wd in w1_dmas[e // 2 + 1]:
                                    tile.add_dep_helper(wd.ins, evac.ins, sync=True,
                                                        reason="w1 prefetch spread across passes")
                        h_sb.append(ht)

                    # Layer 2: Y^T[do] += g * sum_j W2s[j, do]^T Hs^T[j]
                    # y DMAs ride gpsimd; the last two tiles ride scalar then
                    # sync so every queue's drain starts before the barrier.
                    if tile_idx == n_tiles_total - 1:
                        ydma_engines = [nc.sync, nc.scalar]
                    elif tile_idx == n_tiles_total - 2:
                        ydma_engines = [nc.scalar]
                    else:
                        ydma_engines = [nc.gpsimd]
                    for do in range(ND):
                        py = py_pool.tile([128, 512], F32, name=f"py{e}_{t0}_{do}", tag="py")
                        for j in range(NHS):
                            nc.tensor.matmul(
                                py[:, :tn],
                                w2_sb[e][:, j * D + do * 128:j * D + (do + 1) * 128],
                                h_sb[j][:, :tn],
                                start=(j == 0),
                                stop=(j == NHS - 1),
                            )
                        y_sb = ypool.tile([128, 512], F16, name=f"ysb{e}_{t0}_{do}", tag="ysb")
                        nc.vector.tensor_mul(y_sb[:, :tn], py[:, :tn], g_sb[:, :tn])
                        eng = ydma_engines[n_y % len(ydma_engines)]
                        n_y += 1
                        eng.dma_start(yt_ap[do * 128:(do + 1) * 128, t0:t0 + tn], y_sb[:, :tn])
                    tile_idx += 1

    nc.compile()
    return nc


def _route(x, wg, bg):
    """Host router in fp64: per-token top-2 experts and softmax gates."""
    logits = x.astype(np.float64) @ wg.astype(np.float64).T + bg.astype(np.float64)
    top2 = np.argpartition(-logits, 1, axis=1)[:, :TOP_K]  # two largest, unordered
    vals = np.take_along_axis(logits, top2, axis=1)
    ex = np.exp(vals - vals.max(axis=1, keepdims=True))
    gates = ex / ex.sum(axis=1, keepdims=True)
    idxs, gs = [], []
    for e in range(E):
        mask = top2 == e
        rows = np.nonzero(mask.any(axis=1))[0]
        idxs.append(rows)
        gs.append(gates[mask].astype(np.float32))
    return idxs, gs


def moe_run(x, wg, bg, w1, b1, w2, b2, trace=False, trace_kwargs=None):
    x = np.ascontiguousarray(np.asarray(x, np.float32))
    wg = np.asarray(wg, np.float32)
    bg = np.asarray(bg, np.float32)
    w1 = np.asarray(w1, np.float32)
    b1 = np.asarray(b1, np.float32)
    w2 = np.asarray(w2, np.float32)
    b2 = np.asarray(b2, np.float32)
    B = x.shape[0]

    idxs, gs = _route(x, wg, bg)
    counts = [len(r) for r in idxs]
    total = sum(counts)

    nc = build_moe(counts)

    # Shared (identical on every core): sorted activations and gates.
    order = np.concatenate(idxs)
    xt_all = np.ascontiguousarray(x[order].T).astype(np.float16)       # (D, total)
    g_all = np.concatenate(gs).astype(np.float16)                      # (total,)
    g_rep = np.ascontiguousarray(np.broadcast_to(g_all, (128, total)))

    in_maps = []
    for c in range(N_CORES):
        # Core c's H-slice [c*512, (c+1)*512) of every expert.
        w1c = np.concatenate([w1[e][:, c * HS:(c + 1) * HS] for e in range(E)], axis=1)
        w2c = np.concatenate([w2[e][c * HS:(c + 1) * HS, :] for e in range(E)], axis=0)
        b1c = np.concatenate([b1[e][c * HS:(c + 1) * HS].reshape(NHS, 128).T
                              for e in range(E)], axis=1)
        in_maps.append({
            "xt": xt_all,
            "w1": w1c.astype(np.float16),
            "w2": w2c.astype(np.float16),
            "b1": np.ascontiguousarray(b1c),
            "g": g_rep,
        })

    kwargs = {}
    if trace:
        kwargs["trace"] = True
        if trace_kwargs:
            kwargs.update(trace_kwargs)
    res = run_bass_kernel_spmd(nc, in_maps, core_ids=list(range(N_CORES)), **kwargs)

    # Sum the 8 cores' H-slice partials, then scatter-add per-expert segments.
    ysum = res.results[0]["yt"].astype(np.float32)
    for c in range(1, N_CORES):
        ysum += res.results[c]["yt"].astype(np.float32)

    out = np.zeros((B, D), np.float32)
    t = 0
    for e in range(E):
        n = counts[e]
        out[idxs[e]] += ysum[:, t:t + n].T + gs[e][:, None] * b2[e][None, :]
        t += n
    return out, res


def kernel(x, wg, bg, w1, b1, w2, b2):
    out, _ = moe_run(x, wg, bg, w1, b1, w2, b2, trace=False)
    return out
